# revision 1
# baseline (speedup 1.0000x reference)
"""Trainium2 Bass kernel for nn_LooseMatchAssignment.

Math (per batch b):
    s0 = desc0 @ (W0.T @ wc0) + (b0 . wc0)            # [m]
    s1 = desc1 @ (W1.T @ wc1) + (b1 . wc1)            # [n]
    z0 = desc0 @ wm + bm ;  z1 = desc1 @ wm + bm      # [m], [n]
    corres[i,j] = s0_i + s1_j + bc
    probs[i,j]  = ls(corres) + ls(z0_i) + ls(z1_j)
                = ln( sig(corres) * sig(z0_i) * sig(z1_j) )
    probs[i,n] = ln(sig(-z0_i)); probs[m,j] = ln(sig(-z1_j)); probs[m,n] = 0

Sharding: data-parallel over batch, one batch per NeuronCore (8 cores).
Each core writes its full (2049,2049) probs + (2048,2048) corres slice;
host stacks. The kernel is heavily output-DMA-bound (~33.6 MB/core out).

Engine split per 128-row tile ([128, 2048] ops):
    POOL: corres = S1B + a_i       (1-input broadcast add, line rate)
    ACT : sigc = Sigmoid(S1B + a_i)   (corres fused into LUT bias)
    DVE : prod = (sigc * sig(z0_i)) * SG1B       (one scalar_tensor_tensor)
    ACT : inner = Ln(prod)
"""
import sys

if "/opt/trn_rl_repo" not in sys.path:
    sys.path.insert(0, "/opt/trn_rl_repo")

import numpy as np
from contextlib import ExitStack

import concourse.bass as bass
import concourse.bacc as bacc
import concourse.tile as tile
from concourse import mybir, bass_utils

F32 = mybir.dt.float32
AF = mybir.ActivationFunctionType
OP = mybir.AluOpType

B, M, N, K = 8, 2048, 2048, 256
P = 128           # SBUF partitions
T = M // P        # 16 row tiles, side-0 t-major: i = t*P + p
TJ = N // P       # 16, side-1 p-major: j = p*TJ + t

_NC_CACHE: dict = {}


def _bcast(ap, p=P):
    """Broadcast a DRAM AP across p partitions (prepend stride-0 dim)."""
    return bass.AP(tensor=ap.tensor, offset=ap.offset, ap=[[0, p], *ap.ap])


def _free_bcast(ap, count, at=1):
    """Insert a stride-0 free dim of size `count` at position `at` (SBUF AP)."""
    new = list(ap.ap)
    new.insert(at, [0, count])
    return bass.AP(tensor=ap.tensor, offset=ap.offset, ap=new)


def _build_nc(main_bufs=3):
    nc = bacc.Bacc("TRN2", target_bir_lowering=False, debug=False, num_devices=8)
    d0 = nc.dram_tensor("d0", [M, K], F32, kind="ExternalInput")
    d1 = nc.dram_tensor("d1", [N, K], F32, kind="ExternalInput")
    w4 = nc.dram_tensor("w4", [4, K], F32, kind="ExternalInput")
    consts = nc.dram_tensor("consts", [1, 8], F32, kind="ExternalInput")
    probs = nc.dram_tensor("probs", [M + 1, N + 1], F32, kind="ExternalOutput")
    corres = nc.dram_tensor("corres", [M, N], F32, kind="ExternalOutput")

    with tile.TileContext(nc) as tc, ExitStack() as ctx:
        singles = ctx.enter_context(tc.tile_pool(name="singles", bufs=1))
        proj = ctx.enter_context(tc.tile_pool(name="proj", bufs=3))
        work = ctx.enter_context(tc.tile_pool(name="work", bufs=main_bufs))
        dram = ctx.enter_context(tc.tile_pool(name="dram", bufs=1, space="DRAM"))

        # ---- one-time loads ----
        constsB = singles.tile([P, 8], F32)
        nc.sync.dma_start(out=constsB, in_=_bcast(consts.ap()[0:1, :].rearrange("o c -> (o c)")))
        w4B = singles.tile([P, 4, K], F32)
        nc.sync.dma_start(out=w4B, in_=_bcast(w4.ap()))
        # side-1 rows loaded p-major so cols flatten to contiguous DRAM runs
        d1_sb = singles.tile([P, TJ, K], F32)
        nc.sync.dma_start(out=d1_sb, in_=d1.ap().rearrange("(p t) k -> p t k", p=P))
        # side-0 rows t-major: partitions match output row-tiles
        d0_sb = singles.tile([P, T, K], F32)
        nc.sync.dma_start(out=d0_sb, in_=d0.ap().rearrange("(t p) k -> p t k", p=P))

        CA, BM, CS1, NBM = (constsB[:, i : i + 1] for i in range(4))

        # ---- side 1 projections (before side 0: main loop waits on these) ----
        sz1 = singles.tile([P, TJ, 2], F32)
        for t in range(TJ):
            prod = proj.tile([P, 2, K], F32, tag="prj")
            nc.vector.tensor_mul(prod, _free_bcast(d1_sb[:, t, :], 2), w4B[:, 2:4, :])
            nc.vector.reduce_sum(sz1[:, t, :], prod, axis=mybir.AxisListType.X)
        s1_cols = singles.tile([P, TJ], F32)
        nc.scalar.activation(s1_cols, sz1[:, :, 0], AF.Identity, bias=CS1, scale=1.0)
        sg1_cols = singles.tile([P, TJ], F32)
        nc.scalar.activation(sg1_cols, sz1[:, :, 1], AF.Sigmoid, bias=BM, scale=1.0)
        sgm1_cols = singles.tile([P, TJ], F32)
        nc.scalar.activation(sgm1_cols, sz1[:, :, 1], AF.Sigmoid, bias=NBM, scale=-1.0)
        lsm1_cols = singles.tile([P, TJ], F32)
        nc.scalar.activation(lsm1_cols, sgm1_cols, AF.Ln, scale=1.0)

        # flatten p-major cols -> rows in DRAM scratch; broadcast back to all partitions
        rbuf = dram.tile([2, N], F32)
        nc.sync.dma_start(out=rbuf[0:1, :].rearrange("o (p t) -> p (o t)", p=P), in_=s1_cols)
        nc.sync.dma_start(out=rbuf[1:2, :].rearrange("o (p t) -> p (o t)", p=P), in_=sg1_cols)
        # last probs row: ls(-z1_j) straight to DRAM (corner [m,n] stays 0, pre-zeroed)
        nc.sync.dma_start(
            out=probs.ap()[M : M + 1, 0:N].rearrange("o (p t) -> p (o t)", p=P),
            in_=lsm1_cols,
        )
        S1B = singles.tile([P, N], F32)
        nc.sync.dma_start(out=S1B, in_=_bcast(rbuf[0:1, :].rearrange("o n -> (o n)")))
        SG1B = singles.tile([P, N], F32)
        nc.sync.dma_start(out=SG1B, in_=_bcast(rbuf[1:2, :].rearrange("o n -> (o n)")))

        # ---- side 0 projections ----
        sz0 = singles.tile([P, T, 2], F32)
        for t in range(T):
            prod = proj.tile([P, 2, K], F32, tag="prj")
            nc.vector.tensor_mul(prod, _free_bcast(d0_sb[:, t, :], 2), w4B[:, 0:2, :])
            nc.vector.reduce_sum(sz0[:, t, :], prod, axis=mybir.AxisListType.X)
        a_cols = singles.tile([P, T], F32)
        nc.scalar.activation(a_cols, sz0[:, :, 0], AF.Identity, bias=CA, scale=1.0)
        sg0_cols = singles.tile([P, T], F32)
        nc.scalar.activation(sg0_cols, sz0[:, :, 1], AF.Sigmoid, bias=BM, scale=1.0)
        sgm0_cols = singles.tile([P, T], F32)
        nc.scalar.activation(sgm0_cols, sz0[:, :, 1], AF.Sigmoid, bias=NBM, scale=-1.0)
        lsm0_cols = singles.tile([P, T], F32)
        nc.scalar.activation(lsm0_cols, sgm0_cols, AF.Ln, scale=1.0)

        # ---- main loop: one [128, 2048] stripe of corres + probs per iter ----
        for r in range(T):
            a_r = a_cols[:, r : r + 1]
            corres_t = work.tile([P, N], F32, tag="corres")
            nc.gpsimd.tensor_scalar_add(corres_t, S1B, a_r)
            nc.sync.dma_start(out=corres.ap()[r * P : (r + 1) * P, :], in_=corres_t)

            sigc_t = work.tile([P, N], F32, tag="sigc")
            nc.scalar.activation(sigc_t, S1B, AF.Sigmoid, bias=a_r, scale=1.0)
            prod_t = work.tile([P, N], F32, tag="prod")
            nc.vector.scalar_tensor_tensor(
                out=prod_t, in0=sigc_t, scalar=sg0_cols[:, r : r + 1], in1=SG1B,
                op0=OP.mult, op1=OP.mult,
            )
            inner_t = work.tile([P, N + 1], F32, tag="inner")
            nc.scalar.activation(inner_t[:, 0:N], prod_t, AF.Ln, scale=1.0)
            nc.gpsimd.tensor_copy(inner_t[:, N : N + 1], lsm0_cols[:, r : r + 1])
            nc.sync.dma_start(out=probs.ap()[r * P : (r + 1) * P, :], in_=inner_t)

    nc.compile()
    return nc


def _get_nc():
    if "nc" not in _NC_CACHE:
        _NC_CACHE["nc"] = _build_nc()
    return _NC_CACHE["nc"]


def _prep_inputs(desc0, desc1, W0, b0, W1, b1, Wm, bm, Wc, bc):
    desc0 = np.asarray(desc0, dtype=np.float32)
    desc1 = np.asarray(desc1, dtype=np.float32)
    W0 = np.asarray(W0, dtype=np.float64)
    b0 = np.asarray(b0, dtype=np.float64)
    W1 = np.asarray(W1, dtype=np.float64)
    b1 = np.asarray(b1, dtype=np.float64)
    Wm = np.asarray(Wm, dtype=np.float32)
    bm = float(np.asarray(bm).reshape(-1)[0])
    Wc = np.asarray(Wc, dtype=np.float64)
    bc = float(np.asarray(bc).reshape(-1)[0])

    D = W0.shape[0]  # 128
    wc0, wc1 = Wc[0, :D], Wc[0, D:]
    v0 = (W0.T @ wc0).astype(np.float32)
    v1 = (W1.T @ wc1).astype(np.float32)
    c_a = float(b0 @ wc0) + bc
    c_s1 = float(b1 @ wc1)
    w4 = np.stack([v0, Wm[0], v1, Wm[0]]).astype(np.float32)
    consts = np.zeros((1, 8), np.float32)
    consts[0, 0] = c_a
    consts[0, 1] = bm
    consts[0, 2] = c_s1
    consts[0, 3] = -bm

    in_maps = []
    for c in range(B):
        in_maps.append(
            {
                "d0": np.ascontiguousarray(desc0[c]),
                "d1": np.ascontiguousarray(desc1[c]),
                "w4": w4,
                "consts": consts,
            }
        )
    return in_maps


def _run(in_maps, trace=False):
    nc = _get_nc()
    return bass_utils.run_bass_kernel_spmd(
        nc, in_maps, core_ids=list(range(B)), trace=trace
    )


def kernel(desc0, desc1, W0, b0, W1, b1, Wm, bm, Wc, bc):
    assert desc0.shape == (B, M, K) and desc1.shape == (B, N, K), (
        f"unexpected shapes {desc0.shape} {desc1.shape}"
    )
    in_maps = _prep_inputs(desc0, desc1, W0, b0, W1, b1, Wm, bm, Wc, bc)
    res = _run(in_maps)
    probs = np.stack([res.results[c]["probs"] for c in range(B)])
    corres = np.stack([res.results[c]["corres"] for c in range(B)])
    return probs, corres


# revision 3
# speedup vs baseline: 3.0628x; 3.0628x over previous
"""Trainium2 Bass kernel for nn_LooseMatchAssignment.

Math (per batch b):
    s0 = desc0 @ (W0.T @ wc0) + (b0 . wc0)            # [m]
    s1 = desc1 @ (W1.T @ wc1) + (b1 . wc1)            # [n]
    z0 = desc0 @ wm + bm ;  z1 = desc1 @ wm + bm      # [m], [n]
    corres[i,j] = s0_i + s1_j + bc
    probs[i,j]  = ls(corres) + ls(z0_i) + ls(z1_j)
                = ln( sig(corres) * sig(z0_i) * sig(z1_j) )
    probs[i,n] = ln(sig(-z0_i)); probs[m,j] = ln(sig(-z1_j)); probs[m,n] = 0

Sharding: data-parallel over batch, one batch per NeuronCore (8 cores).
Each core writes its full (2049,2049) probs + (2048,2048) corres slice;
host stacks. The kernel is heavily output-DMA-bound (~33.6 MB/core out).

Engine split per 128-row tile ([128, 2048] ops):
    POOL: corres = S1B + a_i       (1-input broadcast add, line rate)
    ACT : sigc = Sigmoid(S1B + a_i)   (corres fused into LUT bias)
    DVE : prod = (sigc * sig(z0_i)) * SG1B       (one scalar_tensor_tensor)
    ACT : inner = Ln(prod)
"""
import sys

if "/opt/trn_rl_repo" not in sys.path:
    sys.path.insert(0, "/opt/trn_rl_repo")

import numpy as np
from contextlib import ExitStack

import concourse.bass as bass
import concourse.bacc as bacc
import concourse.tile as tile
from concourse import mybir, bass_utils

F32 = mybir.dt.float32
AF = mybir.ActivationFunctionType
OP = mybir.AluOpType

B, M, N, K = 8, 2048, 2048, 256
P = 128           # SBUF partitions
T = M // P        # 16 row tiles, side-0 t-major: i = t*P + p
TJ = N // P       # 16, side-1 p-major: j = p*TJ + t

_NC_CACHE: dict = {}


def _bcast(ap, p=P):
    """Broadcast a DRAM AP across p partitions (prepend stride-0 dim)."""
    return bass.AP(tensor=ap.tensor, offset=ap.offset, ap=[[0, p], *ap.ap])


def _free_bcast(ap, count, at=1):
    """Insert a stride-0 free dim of size `count` at position `at` (SBUF AP)."""
    new = list(ap.ap)
    new.insert(at, [0, count])
    return bass.AP(tensor=ap.tensor, offset=ap.offset, ap=new)


def _build_nc(main_bufs=3, group=6, proj_chunk=4):
    nc = bacc.Bacc("TRN2", target_bir_lowering=False, debug=False, num_devices=8)
    d0 = nc.dram_tensor("d0", [M, K], F32, kind="ExternalInput")
    d1 = nc.dram_tensor("d1", [N, K], F32, kind="ExternalInput")
    w4 = nc.dram_tensor("w4", [4, K], F32, kind="ExternalInput")
    consts = nc.dram_tensor("consts", [1, 8], F32, kind="ExternalInput")
    probs = nc.dram_tensor("probs", [M + 1, N + 1], F32, kind="ExternalOutput")
    corres = nc.dram_tensor("corres", [M, N], F32, kind="ExternalOutput")

    with tile.TileContext(nc) as tc, ExitStack() as ctx:
        singles = ctx.enter_context(tc.tile_pool(name="singles", bufs=1))
        proj = ctx.enter_context(tc.tile_pool(name="proj", bufs=2))
        work = ctx.enter_context(tc.tile_pool(name="work", bufs=main_bufs))
        sigp = ctx.enter_context(tc.tile_pool(name="sigp", bufs=group + 2))
        dram = ctx.enter_context(tc.tile_pool(name="dram", bufs=1, space="DRAM"))

        # ---- one-time loads ----
        constsB = singles.tile([P, 8], F32)
        nc.sync.dma_start(out=constsB, in_=_bcast(consts.ap()[0:1, :].rearrange("o c -> (o c)")))
        w4B = singles.tile([P, 4, K], F32)
        nc.sync.dma_start(out=w4B, in_=_bcast(w4.ap()))
        # side-1 rows loaded p-major so cols flatten to contiguous DRAM runs
        d1_sb = singles.tile([P, TJ, K], F32)
        nc.sync.dma_start(out=d1_sb, in_=d1.ap().rearrange("(p t) k -> p t k", p=P))
        # side-0 rows t-major: partitions match output row-tiles
        d0_sb = singles.tile([P, T, K], F32)
        nc.sync.dma_start(out=d0_sb, in_=d0.ap().rearrange("(t p) k -> p t k", p=P))

        CA, BM, CS1, NBM = (constsB[:, i : i + 1] for i in range(4))

        def projections(d_sb, wslice, sz, nt):
            """sz[p, t, v] = sum_k d_sb[p, t, k] * w4B[p, wslice][v, k]; few big DVE ops."""
            for c0 in range(0, nt, proj_chunk):
                cn = min(proj_chunk, nt - c0)
                prodc = proj.tile([P, proj_chunk, 2, K], F32, tag="prj")
                dsl = d_sb[:, c0 : c0 + cn, :]
                in0 = bass.AP(tensor=dsl.tensor, offset=dsl.offset,
                              ap=[dsl.ap[0], dsl.ap[1], [0, 2], dsl.ap[2]])
                wsl = w4B[:, wslice, :]
                in1 = bass.AP(tensor=wsl.tensor, offset=wsl.offset,
                              ap=[wsl.ap[0], [0, cn], wsl.ap[1], wsl.ap[2]])
                nc.vector.tensor_mul(prodc[:, :cn], in0, in1)
                nc.vector.reduce_sum(sz[:, c0 : c0 + cn, :], prodc[:, :cn],
                                     axis=mybir.AxisListType.X)

        # ---- side 1 projections (first: main loop waits on these) ----
        sz1 = singles.tile([P, TJ, 2], F32)
        projections(d1_sb, slice(2, 4), sz1, TJ)
        s1_cols = singles.tile([P, TJ], F32)
        nc.scalar.activation(s1_cols, sz1[:, :, 0], AF.Identity, bias=CS1, scale=1.0)
        sg1_cols = singles.tile([P, TJ], F32)
        nc.scalar.activation(sg1_cols, sz1[:, :, 1], AF.Sigmoid, bias=BM, scale=1.0)
        sgm1_cols = singles.tile([P, TJ], F32)
        nc.scalar.activation(sgm1_cols, sz1[:, :, 1], AF.Sigmoid, bias=NBM, scale=-1.0)
        lsm1_cols = singles.tile([P, TJ], F32)
        nc.scalar.activation(lsm1_cols, sgm1_cols, AF.Ln, scale=1.0)

        # flatten p-major cols -> rows in DRAM scratch; broadcast back to all partitions
        rbuf = dram.tile([2, N], F32)
        nc.sync.dma_start(out=rbuf[0:1, :].rearrange("o (p t) -> p (o t)", p=P), in_=s1_cols)
        nc.sync.dma_start(out=rbuf[1:2, :].rearrange("o (p t) -> p (o t)", p=P), in_=sg1_cols)
        # last probs row: ls(-z1_j) straight to DRAM (corner [m,n] stays 0, pre-zeroed)
        nc.sync.dma_start(
            out=probs.ap()[M : M + 1, 0:N].rearrange("o (p t) -> p (o t)", p=P),
            in_=lsm1_cols,
        )
        S1B = singles.tile([P, N], F32)
        nc.sync.dma_start(out=S1B, in_=_bcast(rbuf[0:1, :].rearrange("o n -> (o n)")))
        SG1B = singles.tile([P, N], F32)
        nc.sync.dma_start(out=SG1B, in_=_bcast(rbuf[1:2, :].rearrange("o n -> (o n)")))

        # ---- side 0 projections ----
        sz0 = singles.tile([P, T, 2], F32)
        projections(d0_sb, slice(0, 2), sz0, T)
        a_cols = singles.tile([P, T], F32)
        nc.scalar.activation(a_cols, sz0[:, :, 0], AF.Identity, bias=CA, scale=1.0)
        sg0_cols = singles.tile([P, T], F32)
        nc.scalar.activation(sg0_cols, sz0[:, :, 1], AF.Sigmoid, bias=BM, scale=1.0)
        sgm0_cols = singles.tile([P, T], F32)
        nc.scalar.activation(sgm0_cols, sz0[:, :, 1], AF.Sigmoid, bias=NBM, scale=-1.0)
        lsm0_cols = singles.tile([P, T], F32)
        nc.scalar.activation(lsm0_cols, sgm0_cols, AF.Ln, scale=1.0)

        # ---- main loop, grouped to batch ACT table swaps (Sigmoid vs Ln) ----
        # per stripe r (rows r*P..r*P+127):
        #   ACT: corres = Id(S1B + a_r)           -> DMA out (sync queue)
        #   ACT: sig = Sigmoid(S1B + a_r)
        #   DVE: sig *= SG1B                       (in place)
        #   ACT: inner = Ln(sig * sg0_r)           (ln(scale)+ln(x) folds sig(z0_i))
        #   POOL: border col = ls(-z0_r); DMA probs stripe (gpsimd queue)
        for g0 in range(0, T, group):
            gn = min(group, T - g0)
            sig_tiles = []
            for r in range(g0, g0 + gn):
                a_r = a_cols[:, r : r + 1]
                corres_t = work.tile([P, N], F32, tag="corres")
                nc.scalar.activation(corres_t, S1B, AF.Identity, bias=a_r, scale=1.0)
                nc.sync.dma_start(out=corres.ap()[r * P : (r + 1) * P, :], in_=corres_t)
                sig_t = sigp.tile([P, N], F32, tag="sig")
                nc.scalar.activation(sig_t, S1B, AF.Sigmoid, bias=a_r, scale=1.0)
                nc.vector.tensor_mul(sig_t, sig_t, SG1B)
                sig_tiles.append(sig_t)
            for i, r in enumerate(range(g0, g0 + gn)):
                inner_t = work.tile([P, N + 1], F32, tag="inner")
                nc.scalar.activation(inner_t[:, 0:N], sig_tiles[i], AF.Ln,
                                     scale=sg0_cols[:, r : r + 1])
                nc.gpsimd.tensor_copy(inner_t[:, N : N + 1], lsm0_cols[:, r : r + 1])
                nc.gpsimd.dma_start(out=probs.ap()[r * P : (r + 1) * P, :], in_=inner_t)

    nc.compile()
    return nc


def _get_nc():
    if "nc" not in _NC_CACHE:
        _NC_CACHE["nc"] = _build_nc()
    return _NC_CACHE["nc"]


def _prep_inputs(desc0, desc1, W0, b0, W1, b1, Wm, bm, Wc, bc):
    desc0 = np.asarray(desc0, dtype=np.float32)
    desc1 = np.asarray(desc1, dtype=np.float32)
    W0 = np.asarray(W0, dtype=np.float64)
    b0 = np.asarray(b0, dtype=np.float64)
    W1 = np.asarray(W1, dtype=np.float64)
    b1 = np.asarray(b1, dtype=np.float64)
    Wm = np.asarray(Wm, dtype=np.float32)
    bm = float(np.asarray(bm).reshape(-1)[0])
    Wc = np.asarray(Wc, dtype=np.float64)
    bc = float(np.asarray(bc).reshape(-1)[0])

    D = W0.shape[0]  # 128
    wc0, wc1 = Wc[0, :D], Wc[0, D:]
    v0 = (W0.T @ wc0).astype(np.float32)
    v1 = (W1.T @ wc1).astype(np.float32)
    c_a = float(b0 @ wc0) + bc
    c_s1 = float(b1 @ wc1)
    w4 = np.stack([v0, Wm[0], v1, Wm[0]]).astype(np.float32)
    consts = np.zeros((1, 8), np.float32)
    consts[0, 0] = c_a
    consts[0, 1] = bm
    consts[0, 2] = c_s1
    consts[0, 3] = -bm

    in_maps = []
    for c in range(B):
        in_maps.append(
            {
                "d0": np.ascontiguousarray(desc0[c]),
                "d1": np.ascontiguousarray(desc1[c]),
                "w4": w4,
                "consts": consts,
            }
        )
    return in_maps


def _run(in_maps, trace=False):
    nc = _get_nc()
    return bass_utils.run_bass_kernel_spmd(
        nc, in_maps, core_ids=list(range(B)), trace=trace
    )


def kernel(desc0, desc1, W0, b0, W1, b1, Wm, bm, Wc, bc):
    assert desc0.shape == (B, M, K) and desc1.shape == (B, N, K), (
        f"unexpected shapes {desc0.shape} {desc1.shape}"
    )
    in_maps = _prep_inputs(desc0, desc1, W0, b0, W1, b1, Wm, bm, Wc, bc)
    res = _run(in_maps)
    probs = np.stack([res.results[c]["probs"] for c in range(B)])
    corres = np.stack([res.results[c]["corres"] for c in range(B)])
    return probs, corres
